# revision 1
# baseline (speedup 1.0000x reference)
"""Trainium2 Bass kernel for nn_InvertedCognitionModel (gnn_message_passing).

Sharding: data-parallel over B=8 across 8 NeuronCores (one batch per core,
weights replicated, no collectives).

Per-core program:
  Phase A: stream x tiles, PE-transpose -> xT tiles, accumulate
      qT = (qW^T xT + qb)/sqrt(KQ), kT = kW^T xT + kb   [32, 2048] f32.
  Merged loop (per 512-wide group of 4 t-tiles):
      sim tile = qT_t^T @ kT [128, 2048] f32; top-8 via DVE max/max_index;
      4 indirect-DMA row-gathers from DRAM x; mean via adds with 0.25 folded
      into the transpose identity -> x1T bf16 (channel-on-partition).
      FFN in transposed layout: a = f1W^T x1T + f1b (bias add in PSUM),
      h'T = a*(1+erf(a/sqrt2)) = 2*gelu(a) bf16;
      y_preT = (0.5 f2W)^T h'T + f2b + x1T (one fused DVE op, bf16);
      LayerNorm: stats by ones-matmul partition reduction, rstd/mu broadcast
      by K=1 matmul, normalize in place; ytopT = t1W_top^T yT + t1b staged
      to DRAM [128, T, 12] bf16 (the scan's precomputed tok contribution).
  Scan: 4 chunks x 512 steps; ytop chunk DMA'd into a ring buffer; inner
      For_i unrolled x8. Vectors live as [128, k] channel-major tiles:
      z = Wbot^T mem_w; a = z + ytop[t]; h = a*(1+erf(a/sqrt2)) bf16;
      prop = (0.5 t2W)^T h + t2b; g = sigmoid(prop); mem += g*(prop-mem).
  Output: out = oW^T memT + ob -> [128, 6]; host reorders to (768,).

Only erf/sigmoid (one ACT table set) run in the scan loop.
"""
import sys
import numpy as np

sys.path.insert(0, "/opt/trn_rl_repo")

import ml_dtypes  # noqa: E402
import concourse.bacc as bacc  # noqa: E402
import concourse.bass as bass  # noqa: E402
import concourse.tile as tile  # noqa: E402
import concourse.mybir as mybir  # noqa: E402
from concourse.bass_utils import run_bass_kernel_spmd  # noqa: E402
from concourse.masks import make_identity  # noqa: E402

B, T, D, KQ, TOPK = 8, 2048, 768, 32, 4
D2 = 2 * D
NT = T // 128          # 16 t-tiles
KD = D // 128          # 6
MF = D2 // 128         # 12
GW = 512               # FFN group width
NG = T // GW           # 4
CH = 512               # scan ytop chunk (steps)
f32 = mybir.dt.float32
bf16 = mybir.dt.bfloat16
u32 = mybir.dt.uint32
AF = mybir.ActivationFunctionType
ALU = mybir.AluOpType
RSQ2 = 0.7071067811865476
bf = ml_dtypes.bfloat16

_CACHE = {}


def build(scan_steps=T, n_cores=8, scan_unroll=8, probe=()):
    # probe: subset of {"static_ytop", "skip_gather", "skip_sim"} for timing
    # experiments only (breaks numerics).
    nc = bacc.Bacc("TRN2", target_bir_lowering=False, debug=False,
                   num_devices=n_cores)

    def din(name, shape, dt=f32):
        return nc.dram_tensor(name, list(shape), dt, kind="ExternalInput").ap()

    xb = din("xb", [T, D])
    qW = din("qW", [D, KQ]); qbs = din("qbs", [KQ, 1])
    kW = din("kW", [D, KQ]); kb = din("kb", [KQ, 1])
    f1W = din("f1W", [D, D2], bf16); f1b = din("f1b", [MF, 128])
    f2Ws = din("f2Ws", [D2, D], bf16); f2b = din("f2b", [KD, 128])
    ln_g = din("ln_g", [KD, 128]); ln_b = din("ln_b", [KD, 128])
    t1TOP = din("t1TOP", [D, D2], bf16); t1b = din("t1b", [MF, 128])
    WBOT = din("WBOT", [D, D2], bf16)
    T2WS = din("T2WS", [D2, D], bf16); t2b = din("t2b", [KD, 128])
    oW = din("oW", [D, D], bf16); ob = din("ob", [KD, 128])
    out_d = nc.dram_tensor("out", [128, KD], f32, kind="ExternalOutput").ap()

    def load_cols(pool, src, n, tag, dt=f32):
        # DRAM [n, 128] -> SBUF [128, n], col k = src[k]
        t_ = pool.tile([128, n], dt, tag=tag, name=tag)
        for k in range(n):
            nc.sync.dma_start(t_[:, k:k + 1], src[k])
        return t_

    with tile.TileContext(nc) as tc, \
            tc.tile_pool(name="dram_scratch", bufs=1, space="DRAM") as dpool:
        ytop_d = dpool.tile([128, T, MF], bf16, tag="ytop")
        # ---------------- Phase A + merged FFN ---------------------------
        with tc.tile_pool(name="persist", bufs=1) as cpool:
            ident = cpool.tile([128, 128], f32)
            make_identity(nc, ident)
            qT = cpool.tile([32, T], f32)
            kT = cpool.tile([32, T], f32)

            with (
                tc.tile_pool(name="pa_w", bufs=1) as pwq,
                tc.tile_pool(name="pa", bufs=3) as pa,
                tc.tile_pool(name="ps_tpA", bufs=2, space="PSUM") as ps_tpA,
                tc.tile_pool(name="ps_qk", bufs=2, space="PSUM") as ps_qk,
            ):
                qW_sb = pwq.tile([128, KD, KQ], f32, tag="qW")
                kW_sb = pwq.tile([128, KD, KQ], f32, tag="kW")
                for k in range(KD):
                    nc.sync.dma_start(qW_sb[:, k], qW[k * 128:(k + 1) * 128, :])
                    nc.sync.dma_start(kW_sb[:, k], kW[k * 128:(k + 1) * 128, :])
                qb_sb = pwq.tile([32, 1], f32, tag="qb")
                kb_sb = pwq.tile([32, 1], f32, tag="kb")
                nc.sync.dma_start(qb_sb[:], qbs[:])
                nc.sync.dma_start(kb_sb[:], kb[:])
                for tt in range(NT):
                    xt = pa.tile([128, D], f32, tag="xt")
                    nc.sync.dma_start(xt[:], xb[tt * 128:(tt + 1) * 128, :])
                    xTt = pa.tile([128, KD, 128], f32, tag="xTt")
                    for kd in range(KD):
                        tp = ps_tpA.tile([128, 128], f32, tag="tp")
                        nc.tensor.transpose(
                            tp[:], xt[:, kd * 128:(kd + 1) * 128], ident[:])
                        (nc.scalar.copy if kd % 2 else nc.vector.tensor_copy)(
                            xTt[:, kd], tp[:])
                    pq = ps_qk.tile([32, 128], f32, tag="pq")
                    for kd in range(KD):
                        nc.tensor.matmul(out=pq[:], lhsT=qW_sb[:, kd],
                                         rhs=xTt[:, kd],
                                         start=(kd == 0), stop=(kd == KD - 1))
                    nc.scalar.activation(qT[:, tt * 128:(tt + 1) * 128], pq[:],
                                         AF.Identity, bias=qb_sb[:],
                                         scale=float(1.0 / np.sqrt(KQ)))
                    pk = ps_qk.tile([32, 128], f32, tag="pk")
                    for kd in range(KD):
                        nc.tensor.matmul(out=pk[:], lhsT=kW_sb[:, kd],
                                         rhs=xTt[:, kd],
                                         start=(kd == 0), stop=(kd == KD - 1))
                    nc.scalar.activation(kT[:, tt * 128:(tt + 1) * 128], pk[:],
                                         AF.Identity, bias=kb_sb[:], scale=1.0)

            # ------- Merged: sim/top-k/gather/FFN/LN/ytop ---------------
            with (
                tc.tile_pool(name="pd_w", bufs=1) as pdw,
                tc.tile_pool(name="pc", bufs=2) as pc,
                tc.tile_pool(name="pcg", bufs=2) as pcg,
                tc.tile_pool(name="pd", bufs=3) as pd,
                tc.tile_pool(name="ps_mm", bufs=2, space="PSUM") as ps_mm,
                tc.tile_pool(name="ps_tp", bufs=2, space="PSUM") as ps_tp,
                tc.tile_pool(name="ps_st", bufs=1, space="PSUM") as ps_st,
                tc.tile_pool(name="ps_bc", bufs=1, space="PSUM") as ps_bc,
            ):
                f1W_sb = pdw.tile([128, KD, D2], bf16, tag="f1W")
                for k in range(KD):
                    nc.sync.dma_start(f1W_sb[:, k], f1W[k * 128:(k + 1) * 128, :])
                f2W_sb = pdw.tile([128, MF, D], bf16, tag="f2W")
                for k in range(MF):
                    nc.sync.dma_start(f2W_sb[:, k], f2Ws[k * 128:(k + 1) * 128, :])
                t1T_sb = pdw.tile([128, KD, D2], bf16, tag="t1T")
                for k in range(KD):
                    nc.sync.dma_start(t1T_sb[:, k], t1TOP[k * 128:(k + 1) * 128, :])
                f1b_sb = load_cols(pdw, f1b, MF, "f1b")
                f2b_sb = load_cols(pdw, f2b, KD, "f2b")
                lng_sb = load_cols(pdw, ln_g, KD, "lng")
                lnb_sb = load_cols(pdw, ln_b, KD, "lnb")
                t1b_sb = load_cols(pdw, t1b, MF, "t1b")
                ones_col = pdw.tile([128, 1], bf16, tag="ones_col")
                nc.vector.memset(ones_col[:], 1.0)
                ones_row = pdw.tile([1, 128], f32, tag="ones_row")
                nc.vector.memset(ones_row[:], 1.0)
                eps_sb = pdw.tile([1, 1], f32, tag="eps")
                nc.vector.memset(eps_sb[:], 1e-5)

                for g in range(NG):
                    x1T = pcg.tile([128, KD, GW], bf16, tag="x1T")
                    for ti in range(GW // 128):
                        tt = g * (GW // 128) + ti
                        sim = pc.tile([128, T], f32, tag="sim")
                        if "skip_sim" in probe:
                            nc.vector.memset(sim[:, :16], 1.0)
                        for sc in ([] if "skip_sim" in probe else range(T // GW)):
                            ps = ps_mm.tile([128, GW], f32, tag="pmm", name="ps")
                            nc.tensor.matmul(
                                out=ps[:], lhsT=qT[:, tt * 128:(tt + 1) * 128],
                                rhs=kT[:, sc * GW:(sc + 1) * GW],
                                start=True, stop=True)
                            nc.scalar.copy(sim[:, sc * GW:(sc + 1) * GW], ps[:])
                        mx = pc.tile([128, 8], f32, tag="mx")
                        nc.vector.max(mx[:], sim[:])
                        idx = pc.tile([128, 8], u32, tag="idx")
                        nc.vector.max_index(idx[:], mx[:], sim[:])
                        gr = [pc.tile([128, D], f32, tag=f"gr{j}",
                                      name=f"gr{j}", bufs=1)
                              for j in range(TOPK)]
                        if "skip_gather" in probe:
                            for j in range(TOPK):
                                nc.vector.memset(gr[j][:], 0.01)
                        else:
                            for _rep in range(8 if "rep_gather" in probe else 1):
                                for j in range(TOPK):
                                    nc.gpsimd.indirect_dma_start(
                                        out=gr[j][:], out_offset=None, in_=xb[:],
                                        in_offset=bass.IndirectOffsetOnAxis(
                                            ap=idx[:, j:j + 1], axis=0))
                        acc = pc.tile([128, D], f32, tag="acc", bufs=1)
                        nc.vector.tensor_add(acc[:], gr[0][:], gr[1][:])
                        x1t = pc.tile([128, D], f32, tag="x1t", bufs=1)
                        nc.vector.tensor_add(x1t[:], gr[2][:], gr[3][:])
                        nc.vector.tensor_add(x1t[:], x1t[:], acc[:])
                        for kd in range(KD):
                            tp = ps_tp.tile([128, 128], f32, tag="tp")
                            nc.tensor.transpose(
                                tp[:], x1t[:, kd * 128:(kd + 1) * 128],
                                ident[:])
                            if kd % 2:
                                nc.scalar.mul(
                                    x1T[:, kd, ti * 128:(ti + 1) * 128],
                                    tp[:], 0.25)
                            else:
                                nc.vector.tensor_scalar_mul(
                                    x1T[:, kd, ti * 128:(ti + 1) * 128],
                                    tp[:], 0.25)
                    # --- FFN mm1 + gelu*2 ---
                    hT = pd.tile([128, MF, GW], bf16, tag="hT", bufs=1)
                    for mf in range(MF):
                        p1 = ps_mm.tile([128, GW], f32, tag="pmm", name="p1")
                        for kd in range(KD):
                            nc.tensor.matmul(
                                out=p1[:],
                                lhsT=f1W_sb[:, kd, mf * 128:(mf + 1) * 128],
                                rhs=x1T[:, kd],
                                start=(kd == 0), stop=(kd == KD - 1))
                        # a = p1 + f1b (in place in PSUM)
                        nc.vector.tensor_scalar_add(p1[:], p1[:],
                                                    f1b_sb[:, mf:mf + 1])
                        e1 = pd.tile([128, GW], f32, tag="e1", bufs=2)
                        nc.scalar.activation(e1[:], p1[:], AF.Erf, scale=RSQ2)
                        nc.vector.scalar_tensor_tensor(
                            hT[:, mf], e1[:], 1.0, p1[:],
                            op0=ALU.add, op1=ALU.mult)
                    # --- FFN mm2 + bias + residual -> y_pre bf16 ---
                    ypT = pd.tile([128, KD, GW], bf16, tag="ypT", bufs=2)
                    sqT = pd.tile([128, KD, GW], bf16, tag="sqT", bufs=1)
                    for kd in range(KD):
                        p2 = ps_mm.tile([128, GW], f32, tag="pmm", name="p2")
                        for mf in range(MF):
                            nc.tensor.matmul(
                                out=p2[:],
                                lhsT=f2W_sb[:, mf, kd * 128:(kd + 1) * 128],
                                rhs=hT[:, mf],
                                start=(mf == 0), stop=(mf == MF - 1))
                        nc.vector.scalar_tensor_tensor(
                            ypT[:, kd], p2[:], f2b_sb[:, kd:kd + 1], x1T[:, kd],
                            op0=ALU.add, op1=ALU.add)
                        nc.vector.tensor_mul(sqT[:, kd], ypT[:, kd], ypT[:, kd])
                    # --- LN stats ---
                    pmu = ps_st.tile([1, GW], f32, tag="pmu")
                    for kd in range(KD):
                        nc.tensor.matmul(out=pmu[:], lhsT=ones_col[:],
                                         rhs=ypT[:, kd],
                                         start=(kd == 0), stop=(kd == KD - 1))
                    pss = ps_st.tile([1, GW], f32, tag="pss")
                    for kd in range(KD):
                        nc.tensor.matmul(out=pss[:], lhsT=ones_col[:],
                                         rhs=sqT[:, kd],
                                         start=(kd == 0), stop=(kd == KD - 1))
                    mu = pd.tile([1, GW], f32, tag="mu", bufs=1)
                    nc.scalar.activation(mu[:], pmu[:], AF.Identity, scale=1.0 / D)
                    var = pd.tile([1, GW], f32, tag="var", bufs=1)
                    nc.scalar.activation(var[:], pss[:], AF.Identity, scale=1.0 / D)
                    mu2 = pd.tile([1, GW], f32, tag="mu2", bufs=1)
                    nc.vector.tensor_mul(mu2[:], mu[:], mu[:])
                    nc.vector.tensor_sub(var[:], var[:], mu2[:])
                    sd = pd.tile([1, GW], f32, tag="sd", bufs=1)
                    nc.scalar.activation(sd[:], var[:], AF.Sqrt, bias=eps_sb[:])
                    rstd = pd.tile([1, GW], f32, tag="rstd", bufs=1)
                    nc.vector.reciprocal(rstd[:], sd[:])
                    nc.vector.tensor_mul(mu[:], mu[:], rstd[:])   # mu*rstd
                    rstd_b = ps_bc.tile([128, GW], f32, tag="rstd_b")
                    murs_b = ps_bc.tile([128, GW], f32, tag="murs_b")
                    nc.tensor.matmul(out=rstd_b[:], lhsT=ones_row[:], rhs=rstd[:],
                                     start=True, stop=True)
                    nc.tensor.matmul(out=murs_b[:], lhsT=ones_row[:], rhs=mu[:],
                                     start=True, stop=True)
                    # --- normalize in place -> yT bf16; ytop matmuls ---
                    for kd in range(KD):
                        nm = pd.tile([128, GW], f32, tag="nm", bufs=2)
                        nc.vector.tensor_mul(nm[:], ypT[:, kd], rstd_b[:])
                        nc.vector.tensor_sub(nm[:], nm[:], murs_b[:])
                        nc.scalar.activation(ypT[:, kd], nm[:], AF.Identity,
                                             bias=lnb_sb[:, kd:kd + 1],
                                             scale=lng_sb[:, kd:kd + 1])
                    ystage = pd.tile([128, GW, MF], bf16, tag="ystage", bufs=2)
                    for mf in range(MF):
                        p3 = ps_mm.tile([128, GW], f32, tag="pmm", name="p3")
                        for kd in range(KD):
                            nc.tensor.matmul(
                                out=p3[:],
                                lhsT=t1T_sb[:, kd, mf * 128:(mf + 1) * 128],
                                rhs=ypT[:, kd],
                                start=(kd == 0), stop=(kd == KD - 1))
                        nc.scalar.activation(ystage[:, :, mf], p3[:],
                                             AF.Identity,
                                             bias=t1b_sb[:, mf:mf + 1])
                    nc.sync.dma_start(ytop_d[:, g * GW:(g + 1) * GW, :],
                                      ystage[:])

        # ---------------- Scan + output ---------------------------------
        with (
            tc.tile_pool(name="pe_w", bufs=1) as pew,
            tc.tile_pool(name="pe_s", bufs=1) as pes,
            tc.tile_pool(name="pe", bufs=2) as pe,
            tc.tile_pool(name="ps_z", bufs=2, space="PSUM") as ps_z,
            tc.tile_pool(name="ps_p2", bufs=2, space="PSUM") as ps_p2,
        ):
            wb_sb = pew.tile([128, KD, D2], bf16, tag="wb")
            for k in range(KD):
                nc.sync.dma_start(wb_sb[:, k], WBOT[k * 128:(k + 1) * 128, :])
            w2_sb = pew.tile([128, MF, D], bf16, tag="w2")
            for k in range(MF):
                nc.sync.dma_start(w2_sb[:, k], T2WS[k * 128:(k + 1) * 128, :])
            t2b_sb = load_cols(pew, t2b, KD, "t2b")
            oW_sb = pew.tile([128, KD, D], bf16, tag="oW")
            for k in range(KD):
                nc.sync.dma_start(oW_sb[:, k], oW[k * 128:(k + 1) * 128, :])
            ob_sb = load_cols(pew, ob, KD, "ob")
            ybuf = pew.tile([128, 2, CH, MF], bf16, tag="ybuf")
            mem = pes.tile([128, KD], f32)
            nc.vector.memset(mem[:], 0.0)
            mem_w = pes.tile([128, KD], bf16)
            nc.vector.memset(mem_w[:], 0.0)

            def step(ybuf_c, iv):
                z = ps_z.tile([128, MF], f32, tag="z", name="z")
                for m in range(MF):
                    for k in range(KD):
                        nc.tensor.matmul(
                            out=z[:, m:m + 1],
                            lhsT=wb_sb[:, k, m * 128:(m + 1) * 128],
                            rhs=mem_w[:, k:k + 1],
                            start=(k == 0), stop=(k == KD - 1))
                a = pe.tile([128, MF], f32, tag="a", name="a")
                ysl = (ybuf_c[:, 0, :] if "static_ytop" in probe
                       else ybuf_c[:, bass.ds(iv, 1), :].squeeze(axis=1))
                nc.vector.tensor_add(a[:], z[:], ysl)
                e = pe.tile([128, MF], f32, tag="e", name="e")
                nc.scalar.activation(e[:], a[:], AF.Erf, scale=RSQ2)
                h = pe.tile([128, MF], bf16, tag="h", name="h")
                nc.vector.scalar_tensor_tensor(
                    h[:], e[:], 1.0, a[:], op0=ALU.add, op1=ALU.mult)
                p2 = ps_p2.tile([128, KD], f32, tag="p2", name="p2")
                for m in range(KD):
                    for k in range(MF):
                        nc.tensor.matmul(
                            out=p2[:, m:m + 1],
                            lhsT=w2_sb[:, k, m * 128:(m + 1) * 128],
                            rhs=h[:, k:k + 1],
                            start=(k == 0), stop=(k == MF - 1))
                prop = pe.tile([128, KD], f32, tag="prop", name="prop")
                nc.vector.tensor_add(prop[:], p2[:], t2b_sb[:])
                gg = pe.tile([128, KD], f32, tag="gg", name="gg")
                nc.scalar.activation(gg[:], prop[:], AF.Sigmoid)
                dd = pe.tile([128, KD], f32, tag="dd", name="dd")
                nc.vector.tensor_sub(dd[:], prop[:], mem[:])
                gd = pe.tile([128, KD], f32, tag="gd", name="gd")
                nc.vector.tensor_mul(gd[:], gg[:], dd[:])
                nc.vector.tensor_add(mem[:], mem[:], gd[:])
                nc.vector.tensor_copy(mem_w[:], mem[:])

            UN = scan_unroll
            n_chunks = (scan_steps + CH - 1) // CH
            NCW = T // CH
            nc.sync.dma_start(ybuf[:, 0], ytop_d[:, 0:CH, :])
            for c in range(n_chunks):
                if c + 1 < n_chunks:
                    cc = (c + 1) % NCW
                    nc.sync.dma_start(
                        ybuf[:, (c + 1) % 2],
                        ytop_d[:, cc * CH:(cc + 1) * CH, :])
                steps_here = min(CH, scan_steps - c * CH)
                with tc.For_i(0, steps_here, UN) as it:
                    iv = nc.snap(it)
                    for u in range(UN):
                        step(ybuf[:, c % 2], iv + u if u else iv)

            osb = pes.tile([128, KD], f32)
            po = ps_z.tile([128, KD], f32, tag="po", name="po")
            for m in range(KD):
                for k in range(KD):
                    nc.tensor.matmul(out=po[:, m:m + 1],
                                     lhsT=oW_sb[:, k, m * 128:(m + 1) * 128],
                                     rhs=mem_w[:, k:k + 1],
                                     start=(k == 0), stop=(k == KD - 1))
                nc.scalar.activation(osb[:, m:m + 1], po[:, m:m + 1],
                                     AF.Identity, bias=ob_sb[:, m:m + 1])
            nc.sync.dma_start(out_d[:], osb[:])

    nc.compile()
    return nc


def _prep(inputs):
    sq = np.float32(1.0 / np.sqrt(KQ))

    def cols(v):  # [n*128] -> [n, 128]
        return np.ascontiguousarray(np.asarray(v, np.float32).reshape(-1, 128))

    base = {
        "qW": np.ascontiguousarray(inputs["qW"], np.float32),
        "qbs": (np.asarray(inputs["qb"], np.float32) * sq).reshape(KQ, 1),
        "kW": np.ascontiguousarray(inputs["kW"], np.float32),
        "kb": np.asarray(inputs["kb"], np.float32).reshape(KQ, 1).copy(),
        "f1W": np.asarray(inputs["f1W"], np.float32).astype(bf),
        "f1b": cols(inputs["f1b"]),
        "f2Ws": (np.asarray(inputs["f2W"], np.float32) * np.float32(0.5)).astype(bf),
        "f2b": cols(inputs["f2b"]),
        "ln_g": cols(inputs["ln_g"]),
        "ln_b": cols(inputs["ln_b"]),
        "t1TOP": np.asarray(inputs["t1W"], np.float32)[:D, :].astype(bf),
        "t1b": cols(inputs["t1b"]),
        "WBOT": np.asarray(inputs["t1W"], np.float32)[D:, :].astype(bf),
        "T2WS": (np.asarray(inputs["t2W"], np.float32) * np.float32(0.5)).astype(bf),
        "t2b": cols(inputs["t2b"]),
        "oW": np.asarray(inputs["oW"], np.float32).astype(bf),
        "ob": cols(inputs["ob"]),
    }
    x = np.asarray(inputs["x"], np.float32)
    return [dict(base, xb=np.ascontiguousarray(x[b])) for b in range(B)]


def kernel(**inputs):
    if "nc" not in _CACHE:
        _CACHE["nc"] = build()
    nc = _CACHE["nc"]
    in_maps = _prep(inputs)
    res = run_bass_kernel_spmd(nc, in_maps, core_ids=list(range(B)))
    outs = []
    for b_ in range(B):
        o = np.asarray(res.results[b_]["out"], np.float32)   # [128, 6]
        outs.append(o.T.reshape(-1))                         # vec[k*128+p]
    return np.stack(outs)

